# revision 3
# baseline (speedup 1.0000x reference)
"""Trainium2 Bass kernel for im2col Conv2d dot-product:
out[b, n] = <enc_x[b, n, :], w_flat> + bias.

Data-parallel over the batch axis: 8 batches per NeuronCore x 8 cores.
Per core: x is [401408, 49] fp32 (~78.7 MB) -> out [401408] fp32.

Layout per core: 7 tiles of [128 partitions, 448 windows, 49] (each
partition holds 448 contiguous windows -> contiguous 87.8 KB DMA rows).
Compute: 49 chained vector-engine ops per tile,
  acc = x[:, :, k] * w[k] + acc   (scalar_tensor_tensor, ping-pong acc),
seeded with k=0 fused as (x*w0 + bias) via tensor_scalar.
Single DVE pass (~180 us) under the HBM DMA roofline (~220 us) -> memory bound.
"""

from contextlib import ExitStack

import numpy as np

import concourse.bass as bass
import concourse.tile as tile
from concourse import mybir

B = 64
WINDOWS = 50176
K = 49
NCORES = 8
BPC = B // NCORES            # batches per core
NWIN = BPC * WINDOWS         # 401408 windows per core
P = 128                      # partitions
W = 448                      # windows per partition per tile
T = NWIN // (P * W)          # 7 tiles
assert T * P * W == NWIN

FP32 = mybir.dt.float32

_NC = None


def _build_nc():
    nc = bass.Bass(trn_type="TRN2", debug=False, num_devices=NCORES)

    x = nc.dram_tensor("x", [NWIN, K], FP32, kind="ExternalInput").ap()
    w = nc.dram_tensor("w", [K], FP32, kind="ExternalInput").ap()
    b = nc.dram_tensor("b", [1], FP32, kind="ExternalInput").ap()
    out = nc.dram_tensor("out", [NWIN], FP32, kind="ExternalOutput").ap()

    x_t = x.rearrange("(t p w) k -> t p w k", t=T, p=P, w=W)
    out_t = out.rearrange("(t p w) -> t p w", t=T, p=P, w=W)

    mult = mybir.AluOpType.mult
    add = mybir.AluOpType.add

    with tile.TileContext(nc) as tc, ExitStack() as ctx:
        consts = ctx.enter_context(tc.tile_pool(name="consts", bufs=1))
        xpool = ctx.enter_context(tc.tile_pool(name="x", bufs=2))
        apool = ctx.enter_context(tc.tile_pool(name="acc", bufs=4))

        # Broadcast the 49 weights and the bias to all 128 partitions.
        wb = consts.tile([P, K], FP32)
        nc.gpsimd.dma_start(
            out=wb[:],
            in_=bass.AP(tensor=w.tensor, offset=w.offset, ap=[[0, P]] + list(w.ap)),
        )
        bb = consts.tile([P, 1], FP32)
        nc.gpsimd.dma_start(
            out=bb[:],
            in_=bass.AP(tensor=b.tensor, offset=b.offset, ap=[[0, P]] + list(b.ap)),
        )

        for t in range(T):
            xt = xpool.tile([P, W, K], FP32, tag="xt")
            nc.sync.dma_start(out=xt[:], in_=x_t[t])

            acc_a = apool.tile([P, W], FP32, tag="acc")
            acc_b = apool.tile([P, W], FP32, tag="acc")

            # k = 0: acc = x0 * w[0] + bias
            nc.vector.tensor_scalar(
                out=acc_a[:],
                in0=xt[:, :, 0],
                scalar1=wb[:, 0:1],
                scalar2=bb[:, 0:1],
                op0=mult,
                op1=add,
            )
            cur, nxt = acc_a, acc_b
            for k in range(1, K):
                nc.vector.scalar_tensor_tensor(
                    out=nxt[:],
                    in0=xt[:, :, k],
                    scalar=wb[:, k : k + 1],
                    in1=cur[:],
                    op0=mult,
                    op1=add,
                )
                cur, nxt = nxt, cur

            nc.sync.dma_start(out=out_t[t], in_=cur[:])

    return nc


def _split_ctrl_waits(nc, max_waits=1):
    """Work around a walrus codegen limit: CTRL-type instructions (Drain/NoOp)
    on this build accept only one sync-wait command. Tile's kernel-tail drain
    waits on every pending DMA-completion semaphore at once, which trips
    "Too many sync wait commands". Hoist the extra waits onto dedicated no-op
    instructions inserted just before, preserving per-engine order."""
    from concourse import mybir

    for f in nc.m.functions:
        for blk in f.blocks:
            insts = blk.instructions
            i = 0
            while i < len(insts):
                ins = insts[i]
                if (
                    ins.sync_info is not None
                    and len(ins.sync_info.on_wait) > max_waits
                ):
                    waits = list(ins.sync_info.on_wait)
                    keep, extra = waits[:max_waits], waits[max_waits:]
                    ins.sync_info.on_wait = keep
                    for j, wchunk in enumerate(extra):
                        nop = mybir.InstNoOp(
                            name=f"{ins.name}-wsplit{j}",
                            sync_info=mybir.SyncInfo(on_wait=[wchunk], on_update=[]),
                            bass_nofuse=True,
                            engine=ins.engine,
                        )
                        nc.register_instruction(nop, overwrite=True)
                        insts.insert(i, nop)
                        i += 1
                i += 1


def _get_nc():
    global _NC
    if _NC is None:
        _NC = _build_nc()
        _split_ctrl_waits(_NC)
    return _NC


def run(enc_x, weight, bias, trace=False, **spmd_kwargs):
    """Run on 8 NeuronCores; returns (out [B, WINDOWS] fp32, BassKernelResults)."""
    from concourse.bass_utils import run_bass_kernel_spmd

    nc = _get_nc()
    xf = np.ascontiguousarray(np.asarray(enc_x), dtype=np.float32).reshape(
        NCORES, NWIN, K
    )
    wf = np.ascontiguousarray(np.asarray(weight), dtype=np.float32).reshape(K)
    bf = np.ascontiguousarray(np.asarray(bias), dtype=np.float32).reshape(1)
    in_maps = [{"x": xf[i], "w": wf, "b": bf} for i in range(NCORES)]
    res = run_bass_kernel_spmd(
        nc, in_maps, list(range(NCORES)), trace=trace, **spmd_kwargs
    )
    out = np.stack([res.results[i]["out"] for i in range(NCORES)], axis=0)
    return out.reshape(B, WINDOWS), res


def kernel(enc_x, weight, bias, windows_nb=None):
    out, _ = run(enc_x, weight, bias)
    return out


# revision 4
# speedup vs baseline: 1.0344x; 1.0344x over previous
"""Trainium2 Bass kernel for im2col Conv2d dot-product:
out[b, n] = <enc_x[b, n, :], w_flat> + bias.

Data-parallel over batch: 8 batches per NeuronCore x 8 cores.
Per core: x is [401408, 49] fp32 (~78.7 MB) -> out [401408] fp32.
Memory-bound: HBM roofline ~220 us/core at ~358 GB/s.

Per tile [128, W, 49] (partition p holds W contiguous windows):
  1. in-place multiply x *= w_bcast  (one big contiguous op; the weight
     operand is a [128, W, 49] stride-0-broadcast view of a [128, 49] tile)
  2. segmented sum: tensor_reduce axis=X -> [128, W]   (DVE, 1.0 cyc/elem)
  3. + bias (tensor_scalar, 2x mode), DMA out.
The multiply is spread across engines so no engine exceeds the DMA time:
DVE does all reduces (~163 us) + 2 tile multiplies, GpSimd does most
multiplies (1.68 ns/elem), ScalarE does 2 tiles as 49 strided per-k
activation-muls. Tail tiles are small (W=49) to cut the end-of-stream
latency after the last DMA.
"""

from contextlib import ExitStack

import numpy as np

import concourse.bass as bass
import concourse.tile as tile
from concourse import mybir

B = 64
WINDOWS = 50176
K = 49
NCORES = 8
BPC = B // NCORES            # batches per core
NWIN = BPC * WINDOWS         # 401408 windows per core
P = 128                      # partitions

WBIG = 196                   # windows per partition, big tiles
WSMALL = 49                  # windows per partition, tail tiles
TBIG = 15
TSMALL = 4
assert TBIG * P * WBIG + TSMALL * P * WSMALL == NWIN

# Multiply-engine assignment for big tiles (index in 0..TBIG-1):
DVE_MULT = {6, 13}           # 2 big tiles multiplied on vector engine
SCE_MULT = {2, 9}            # 2 big tiles multiplied on scalar engine

FP32 = mybir.dt.float32

_NC = None


def _build_nc():
    nc = bass.Bass(trn_type="TRN2", debug=False, num_devices=NCORES)

    x = nc.dram_tensor("x", [NWIN, K], FP32, kind="ExternalInput").ap()
    w = nc.dram_tensor("w", [K], FP32, kind="ExternalInput").ap()
    b = nc.dram_tensor("b", [1], FP32, kind="ExternalInput").ap()
    out = nc.dram_tensor("out", [NWIN], FP32, kind="ExternalOutput").ap()

    mult = mybir.AluOpType.mult
    add = mybir.AluOpType.add

    with tile.TileContext(nc) as tc, ExitStack() as ctx:
        consts = ctx.enter_context(tc.tile_pool(name="consts", bufs=1))
        xpool = ctx.enter_context(tc.tile_pool(name="x", bufs=3))
        opool = ctx.enter_context(tc.tile_pool(name="o", bufs=4))

        wb = consts.tile([P, K], FP32)
        nc.gpsimd.dma_start(
            out=wb[:],
            in_=bass.AP(tensor=w.tensor, offset=w.offset, ap=[[0, P]] + list(w.ap)),
        )
        bb = consts.tile([P, 1], FP32)
        nc.gpsimd.dma_start(
            out=bb[:],
            in_=bass.AP(tensor=b.tensor, offset=b.offset, ap=[[0, P]] + list(b.ap)),
        )
        wb_ap = wb[:]

        def w_bcast(wn):
            # [P, wn, K] stride-0-broadcast view of the [P, K] weights tile
            return bass.AP(
                tensor=wb_ap.tensor,
                offset=wb_ap.offset,
                ap=[wb_ap.ap[0], [0, wn], wb_ap.ap[1]],
            )

        def do_tile(win_base, wn, mult_engine, name):
            xt = xpool.tile([P, wn, K], FP32, tag="xt", name=f"xt{name}")
            # partition p <- windows [win_base + p*wn, win_base + (p+1)*wn)
            src = bass.AP(
                tensor=x.tensor,
                offset=x.offset + win_base * K,
                ap=[[wn * K, P], [1, wn * K]],
            )
            nc.sync.dma_start(out=xt[:].rearrange("p w k -> p (w k)"), in_=src)

            if mult_engine == "scalar":
                # 49 strided in-place per-k muls: x[:, :, k] *= w[k]
                for k in range(K):
                    nc.scalar.mul(
                        out=xt[:, :, k], in_=xt[:, :, k], mul=wb_ap[:, k : k + 1]
                    )
            else:
                eng = nc.vector if mult_engine == "vector" else nc.gpsimd
                eng.tensor_tensor(
                    out=xt[:], in0=xt[:], in1=w_bcast(wn), op=mult
                )

            pre = opool.tile([P, wn], FP32, tag="pre", name=f"pre{name}")
            nc.vector.tensor_reduce(
                out=pre[:], in_=xt[:], axis=mybir.AxisListType.X, op=add
            )
            acc = opool.tile([P, wn], FP32, tag="acc", name=f"acc{name}")
            nc.vector.tensor_scalar(
                out=acc[:], in0=pre[:], scalar1=bb[:, 0:1], scalar2=None, op0=add
            )
            dst = bass.AP(
                tensor=out.tensor,
                offset=out.offset + win_base,
                ap=[[wn, P], [1, wn]],
            )
            nc.sync.dma_start(out=dst, in_=acc[:])

        base = 0
        for t in range(TBIG):
            eng = (
                "vector" if t in DVE_MULT else
                "scalar" if t in SCE_MULT else
                "gpsimd"
            )
            do_tile(base, WBIG, eng, f"b{t}")
            base += P * WBIG
        for t in range(TSMALL):
            do_tile(base, WSMALL, "gpsimd", f"s{t}")
            base += P * WSMALL
        assert base == NWIN

    return nc


def _split_ctrl_waits(nc, max_waits=1):
    """Work around a walrus codegen limit on this build: instructions accept
    only one sync-wait command. Hoist extra waits onto dedicated no-op
    instructions inserted just before, preserving per-engine order."""
    from concourse import mybir

    for f in nc.m.functions:
        for blk in f.blocks:
            insts = blk.instructions
            i = 0
            while i < len(insts):
                ins = insts[i]
                if (
                    ins.sync_info is not None
                    and len(ins.sync_info.on_wait) > max_waits
                ):
                    waits = list(ins.sync_info.on_wait)
                    keep, extra = waits[:max_waits], waits[max_waits:]
                    ins.sync_info.on_wait = keep
                    for j, wchunk in enumerate(extra):
                        nop = mybir.InstNoOp(
                            name=f"{ins.name}-wsplit{j}",
                            sync_info=mybir.SyncInfo(on_wait=[wchunk], on_update=[]),
                            bass_nofuse=True,
                            engine=ins.engine,
                        )
                        nc.register_instruction(nop, overwrite=True)
                        insts.insert(i, nop)
                        i += 1
                i += 1


def _get_nc():
    global _NC
    if _NC is None:
        _NC = _build_nc()
        _split_ctrl_waits(_NC)
    return _NC


def run(enc_x, weight, bias, trace=False, **spmd_kwargs):
    """Run on 8 NeuronCores; returns (out [B, WINDOWS] fp32, BassKernelResults)."""
    from concourse.bass_utils import run_bass_kernel_spmd

    nc = _get_nc()
    xf = np.ascontiguousarray(np.asarray(enc_x), dtype=np.float32).reshape(
        NCORES, NWIN, K
    )
    wf = np.ascontiguousarray(np.asarray(weight), dtype=np.float32).reshape(K)
    bf = np.ascontiguousarray(np.asarray(bias), dtype=np.float32).reshape(1)
    in_maps = [{"x": xf[i], "w": wf, "b": bf} for i in range(NCORES)]
    res = run_bass_kernel_spmd(
        nc, in_maps, list(range(NCORES)), trace=trace, **spmd_kwargs
    )
    out = np.stack([res.results[i]["out"] for i in range(NCORES)], axis=0)
    return out.reshape(B, WINDOWS), res


def kernel(enc_x, weight, bias, windows_nb=None):
    out, _ = run(enc_x, weight, bias)
    return out


# revision 5
# speedup vs baseline: 1.1705x; 1.1316x over previous
"""Trainium2 Bass kernel for im2col Conv2d dot-product:
out[b, n] = <enc_x[b, n, :], w_flat> + bias.

Data-parallel over batch: 8 batches per NeuronCore x 8 cores.
Per core: x is [401408, 49] fp32 (~78.7 MB) -> out [401408] fp32.
Memory-bound: HBM roofline ~220 us/core at ~358 GB/s.

Per tile [128, W, 49] (partition p holds W contiguous windows):
  1. in-place multiply x *= w_bcast  (one big contiguous op; the weight
     operand is a [128, W, 49] stride-0-broadcast view of a [128, 49] tile)
  2. segmented sum: tensor_reduce axis=X -> [128, W]   (DVE, 1.0 cyc/elem)
  3. + bias (tensor_scalar, 2x mode), DMA out.
The multiply is spread across engines so no engine exceeds the DMA time:
DVE does all reduces (~163 us) + 2 tile multiplies, GpSimd does most
multiplies (1.68 ns/elem), ScalarE does 2 tiles as 49 strided per-k
activation-muls. Tail tiles are small (W=49) to cut the end-of-stream
latency after the last DMA.
"""

from contextlib import ExitStack

import numpy as np

import concourse.bass as bass
import concourse.tile as tile
from concourse import mybir

B = 64
WINDOWS = 50176
K = 49
NCORES = 8
BPC = B // NCORES            # batches per core
NWIN = BPC * WINDOWS         # 401408 windows per core
P = 128                      # partitions

WBIG = 196                   # windows per partition, big tiles
WSMALL = 49                  # windows per partition, tail tiles
TBIG = 15
TSMALL = 4
assert TBIG * P * WBIG + TSMALL * P * WSMALL == NWIN

# Multiply-engine assignment for big tiles (index in 0..TBIG-1):
# DVE takes 5 of 15 big-tile multiplies (it also does every reduce);
# GpSimd takes the rest. ScalarE only does the cheap contiguous bias-add
# (its strided per-k multiply measured 36.5us/tile -- far too slow).
DVE_MULT = {1, 4, 7, 10, 13}

FP32 = mybir.dt.float32

_NC = None


def _build_nc():
    nc = bass.Bass(trn_type="TRN2", debug=False, num_devices=NCORES)

    x = nc.dram_tensor("x", [NWIN, K], FP32, kind="ExternalInput").ap()
    w = nc.dram_tensor("w", [K], FP32, kind="ExternalInput").ap()
    b = nc.dram_tensor("b", [1], FP32, kind="ExternalInput").ap()
    out = nc.dram_tensor("out", [NWIN], FP32, kind="ExternalOutput").ap()

    mult = mybir.AluOpType.mult
    add = mybir.AluOpType.add

    with tile.TileContext(nc) as tc, ExitStack() as ctx:
        consts = ctx.enter_context(tc.tile_pool(name="consts", bufs=1))
        xpool = ctx.enter_context(tc.tile_pool(name="x", bufs=4))
        opool = ctx.enter_context(tc.tile_pool(name="o", bufs=4))

        wb = consts.tile([P, K], FP32)
        nc.gpsimd.dma_start(
            out=wb[:],
            in_=bass.AP(tensor=w.tensor, offset=w.offset, ap=[[0, P]] + list(w.ap)),
        )
        bb = consts.tile([P, 1], FP32)
        nc.gpsimd.dma_start(
            out=bb[:],
            in_=bass.AP(tensor=b.tensor, offset=b.offset, ap=[[0, P]] + list(b.ap)),
        )
        wb_ap = wb[:]

        def w_bcast(wn):
            # [P, wn, K] stride-0-broadcast view of the [P, K] weights tile
            return bass.AP(
                tensor=wb_ap.tensor,
                offset=wb_ap.offset,
                ap=[wb_ap.ap[0], [0, wn], wb_ap.ap[1]],
            )

        def do_tile(win_base, wn, mult_engine, name):
            xt = xpool.tile([P, wn, K], FP32, tag="xt", name=f"xt{name}")
            # partition p <- windows [win_base + p*wn, win_base + (p+1)*wn)
            src = bass.AP(
                tensor=x.tensor,
                offset=x.offset + win_base * K,
                ap=[[wn * K, P], [1, wn * K]],
            )
            nc.sync.dma_start(out=xt[:].rearrange("p w k -> p (w k)"), in_=src)

            eng = nc.vector if mult_engine == "vector" else nc.gpsimd
            eng.tensor_tensor(out=xt[:], in0=xt[:], in1=w_bcast(wn), op=mult)

            pre = opool.tile([P, wn], FP32, tag="pre", name=f"pre{name}")
            nc.vector.tensor_reduce(
                out=pre[:], in_=xt[:], axis=mybir.AxisListType.X, op=add
            )
            acc = opool.tile([P, wn], FP32, tag="acc", name=f"acc{name}")
            # bias add on the (otherwise idle) scalar engine, contiguous 1x
            nc.scalar.activation(
                out=acc[:], in_=pre[:],
                func=mybir.ActivationFunctionType.Identity,
                bias=bb[:, 0:1], scale=1.0,
            )
            dst = bass.AP(
                tensor=out.tensor,
                offset=out.offset + win_base,
                ap=[[wn, P], [1, wn]],
            )
            nc.sync.dma_start(out=dst, in_=acc[:])

        base = 0
        for t in range(TBIG):
            eng = "vector" if t in DVE_MULT else "gpsimd"
            do_tile(base, WBIG, eng, f"b{t}")
            base += P * WBIG
        for t in range(TSMALL):
            do_tile(base, WSMALL, "gpsimd", f"s{t}")
            base += P * WSMALL
        assert base == NWIN

    return nc


def _split_ctrl_waits(nc, max_waits=1):
    """Work around a walrus codegen limit on this build: instructions accept
    only one sync-wait command. Hoist extra waits onto dedicated no-op
    instructions inserted just before, preserving per-engine order."""
    from concourse import mybir

    for f in nc.m.functions:
        for blk in f.blocks:
            insts = blk.instructions
            i = 0
            while i < len(insts):
                ins = insts[i]
                if (
                    ins.sync_info is not None
                    and len(ins.sync_info.on_wait) > max_waits
                ):
                    waits = list(ins.sync_info.on_wait)
                    keep, extra = waits[:max_waits], waits[max_waits:]
                    ins.sync_info.on_wait = keep
                    for j, wchunk in enumerate(extra):
                        nop = mybir.InstNoOp(
                            name=f"{ins.name}-wsplit{j}",
                            sync_info=mybir.SyncInfo(on_wait=[wchunk], on_update=[]),
                            bass_nofuse=True,
                            engine=ins.engine,
                        )
                        nc.register_instruction(nop, overwrite=True)
                        insts.insert(i, nop)
                        i += 1
                i += 1


def _get_nc():
    global _NC
    if _NC is None:
        _NC = _build_nc()
        _split_ctrl_waits(_NC)
    return _NC


def run(enc_x, weight, bias, trace=False, **spmd_kwargs):
    """Run on 8 NeuronCores; returns (out [B, WINDOWS] fp32, BassKernelResults)."""
    from concourse.bass_utils import run_bass_kernel_spmd

    nc = _get_nc()
    xf = np.ascontiguousarray(np.asarray(enc_x), dtype=np.float32).reshape(
        NCORES, NWIN, K
    )
    wf = np.ascontiguousarray(np.asarray(weight), dtype=np.float32).reshape(K)
    bf = np.ascontiguousarray(np.asarray(bias), dtype=np.float32).reshape(1)
    in_maps = [{"x": xf[i], "w": wf, "b": bf} for i in range(NCORES)]
    res = run_bass_kernel_spmd(
        nc, in_maps, list(range(NCORES)), trace=trace, **spmd_kwargs
    )
    out = np.stack([res.results[i]["out"] for i in range(NCORES)], axis=0)
    return out.reshape(B, WINDOWS), res


def kernel(enc_x, weight, bias, windows_nb=None):
    out, _ = run(enc_x, weight, bias)
    return out
